# revision 1
# baseline (speedup 1.0000x reference)
"""Trainium2 Bass kernel for nn_DescriptorModuleSpecies (gnn_message_passing).

Sharding: data-parallel, one snapshot per NeuronCore (8 cores).

Algorithmic core (exact algebra of the reference):
    D[n] = Q[n]^T @ Q[n][:, :16],   Q[n] = sum_m r_tilde(n,m) ⊗ G(s(n,m), pair)
The species-pair MLPs (es/fs) and en1/en2 are folded on the host into an
exact piecewise-linear basis in s per species-pair class:
    G(s; class) = sum_beta phi_beta(s) * W3''[beta, :]      (W ~= 54 basis fns)
Per-edge basis planes cost one fused DVE/ACT op each; per-atom moments
Phi[d, beta] = sum_m r_tilde_d * phi_beta are computed with one small PE
matmul per atom-pair column (contraction over the 128 edge rows of the
(2 atoms x 64 slots) layout), then Q = Phi @ W3'' and D via per-atom
broadcast multiplies on DVE.

Neighbor gather: gpsimd ap_gather from an SBUF-resident interleaved table
(partition p holds component p%4 of (x, y, z, type)), per-Q7-core index
streams prepared on the host (pure index-layout preprocessing), followed by
SBUF->SBUF DMAs splitting component rows into edge planes.
"""

import sys

import numpy as np

try:
    import concourse.bass as bass  # noqa: F401
except Exception:  # pragma: no cover
    sys.path.insert(0, "/opt/trn_rl_repo")

import concourse.bass as bass
import concourse.bacc as bacc
import concourse.mybir as mybir
from concourse.bass_utils import run_bass_kernel_spmd
from concourse.tile import TileContext

F32 = mybir.dt.float32
I32 = mybir.dt.int32
I16 = mybir.dt.int16
AF = mybir.ActivationFunctionType
ALU = mybir.AluOpType

S, N, M = 8, 4096, 64
L = 20.0
JTOT = N // 2              # 2048 atom-pair columns
NCHUNK = 8
JC = JTOT // NCHUNK        # 256 cols per chunk
NI = 16 * JC               # ap_gather num_idxs per Q7 core per chunk
NCORES = 8
SUBJ = 128                 # moment sub-chunk (j columns per bas tile)

CLASSES = [(0, 0), (0, 1), (1, 1)]   # pair (0,1) == (1,0) exactly (symmetrized)


def _mlp_np(x, params):
    n = len(params)
    for i, (w, b) in enumerate(params):
        x = x @ w + b
        if i < n - 1:
            x = np.maximum(x, 0.0)
    return x


def _fold_weights(ws):
    """Exact PL basis for h2(s; class) folded with en3 into W3''.

    Returns (basis, w3pp): basis is a list of ("one"|"lin"|"relu", cls, knot);
    w3pp [W, 32] f32 with G_edge = sum_beta basis_beta * w3pp[beta]."""
    es = [(ws["es1_w"], ws["es1_b"]), (ws["es2_w"], ws["es2_b"])]
    fs = [(ws["fs1_w"], ws["fs1_b"]), (ws["fs2_w"], ws["fs2_b"])]
    W1, b1 = ws["en1_w"].astype(np.float64), ws["en1_b"].astype(np.float64)
    W2, b2 = ws["en2_w"].astype(np.float64), ws["en2_b"].astype(np.float64)
    W3, b3 = ws["en3_w"].astype(np.float64), ws["en3_b"].astype(np.float64)

    basis, psis = [], []
    for ci, (a, b) in enumerate(CLASSES):
        pair = np.array([[a, b]], dtype=np.float32)
        td = _mlp_np(_mlp_np(pair, es) + _mlp_np(pair[:, ::-1], es), fs)[0]
        td = td.astype(np.float64)
        U = td @ W1                                   # [8]

        def h2_of(s):
            h1 = np.maximum(np.outer(s, U) + b1[None, :], 0.0)
            return np.maximum(h1 @ W2 + b2[None, :], 0.0)

        kn1 = sorted(float(-b1[c] / U[c]) for c in range(8)
                     if U[c] != 0.0 and -b1[c] / U[c] > 0.0)
        segpts = [0.0] + kn1
        cross = set()
        for i in range(len(segpts)):
            lo = segpts[i]
            hi = segpts[i + 1] if i + 1 < len(segpts) else None
            mid = (lo + hi) / 2 if hi is not None else lo + 1.0
            act = (mid * U + b1) > 0
            z_lo = np.maximum(lo * U + b1, 0.0) @ W2 + b2
            slope = (U * act) @ W2
            for f in range(16):
                if slope[f] == 0.0:
                    continue
                t = lo - z_lo[f] / slope[f]
                if t > lo and (hi is None or t < hi) and t > 0.0:
                    cross.add(float(t))
        knots = sorted(set(kn1) | cross)

        def seg_slope(lo, hi):
            mid = (lo + hi) / 2 if hi is not None else lo + 1.0
            act1 = (mid * U + b1) > 0
            z_mid = np.maximum(mid * U + b1, 0.0) @ W2 + b2
            return ((U * act1) @ W2) * (z_mid > 0)

        alpha = h2_of(np.array([0.0]))[0]
        bounds = knots + [None]
        slopes = [seg_slope(0.0 if i == 0 else knots[i - 1], bounds[i])
                  for i in range(len(knots) + 1)]
        basis.append(("one", ci, 0.0)); psis.append(alpha)
        basis.append(("lin", ci, 0.0)); psis.append(slopes[0])
        for i, t in enumerate(knots):
            basis.append(("relu", ci, float(t)))
            psis.append(slopes[i + 1] - slopes[i])

    Psi = np.stack(psis, 0)
    w3pp = Psi @ W3
    for i, (kind, ci, t) in enumerate(basis):
        if kind == "one":
            w3pp[i] += b3
    return basis, w3pp.astype(np.float32)


def _verify_fold(ws, basis, w3pp):
    es = [(ws["es1_w"], ws["es1_b"]), (ws["es2_w"], ws["es2_b"])]
    fs = [(ws["fs1_w"], ws["fs1_b"]), (ws["fs2_w"], ws["fs2_b"])]
    rng = np.random.default_rng(0)
    sv = np.concatenate([rng.uniform(0, 5, 64), rng.uniform(0, 1000, 32), [0.0]])
    for ci, (a, b) in enumerate(CLASSES):
        pair = np.array([[a, b]], dtype=np.float32)
        td = _mlp_np(_mlp_np(pair, es) + _mlp_np(pair[:, ::-1], es), fs)[0]
        st = sv[:, None] * td[None, :].astype(np.float64)
        G = _mlp_np(st, [(ws["en1_w"], ws["en1_b"]), (ws["en2_w"], ws["en2_b"]),
                         (ws["en3_w"], ws["en3_b"])])
        vals = np.zeros((len(sv), len(basis)))
        for i, (kind, cc, t) in enumerate(basis):
            if cc != ci:
                continue
            vals[:, i] = 1.0 if kind == "one" else (sv if kind == "lin"
                                                    else np.maximum(sv - t, 0.0))
        Gb = vals @ w3pp.astype(np.float64)
        err = np.abs(Gb - G).max() / (np.abs(G).max() + 1e-9)
        assert err < 1e-4, f"basis fold mismatch class {ci}: rel {err}"


def _reg_consts(nc, vals):
    for v in vals:
        key = (F32, float(v))
        if key in nc.const_aps.aps:
            continue
        t = nc.alloc_sbuf_tensor(f"constf32_{len(nc.const_aps.aps)}", [128, 1], F32)
        nc.gpsimd.memset(t.ap(), float(v))
        nc.const_aps.aps[key] = t.ap()
    nc.all_engine_barrier()


def _build_program(basis):
    Wb = len(basis)
    assert Wb <= 128
    # engine split for basis relu planes: alternate DVE / ACT
    act_knots = sorted({t for k, c, t in basis if k == "relu"})

    nc = bacc.Bacc("TRN2", target_bir_lowering=False, debug=False,
                   num_devices=NCORES)
    _reg_consts(nc, [0.0, 1e-12, float(np.pi)] + [-t for t in act_knots])

    table = nc.dram_tensor("table", [128, N], F32, kind="ExternalInput")
    idxw = nc.dram_tensor("idxw", [128, JTOT], I16, kind="ExternalInput")
    nqd = nc.dram_tensor("nq", [128, JTOT], I32, kind="ExternalInput")
    xi = nc.dram_tensor("xi", [128, JTOT], F32, kind="ExternalInput")
    yi = nc.dram_tensor("yi", [128, JTOT], F32, kind="ExternalInput")
    zi = nc.dram_tensor("zi", [128, JTOT], F32, kind="ExternalInput")
    ai = nc.dram_tensor("ai", [128, JTOT], F32, kind="ExternalInput")
    w3t = nc.dram_tensor("w3pp", [Wb, 32], F32, kind="ExternalInput")
    dout = nc.dram_tensor("dout", [N, 512], F32, kind="ExternalOutput")

    with TileContext(nc) as tc:
        with (
            tc.tile_pool(name="persist", bufs=1) as pp,
            tc.tile_pool(name="work", bufs=2) as wp,
            tc.tile_pool(name="bas", bufs=1) as bp,
            tc.tile_pool(name="psum", bufs=4, space="PSUM") as psp,
            tc.tile_pool(name="qpsum", bufs=4, space="PSUM") as qsp,
        ):
            tab = pp.tile([128, N], F32)
            nc.sync.dma_start(tab[:], table[:])
            w3s = pp.tile([Wb, 32], F32)
            nc.sync.dma_start(w3s[:], w3t[:])
            qt = pp.tile([128, 128 * 32], F32)     # [(8j16+4q+d), 32*grp + g]
            q2 = pp.tile([128, 4096], F32)         # [atom%128, 128*t + 32*d + g]

            for c in range(NCHUNK):
                j0 = c * JC
                idx = wp.tile([128, JC], I16, tag="idx")
                nc.sync.dma_start(idx[:], idxw[:, j0:j0 + JC])
                nqc = wp.tile([128, JC], I32, tag="nqc")
                nc.sync.dma_start(nqc[:], nqd[:, j0:j0 + JC])
                xic = wp.tile([128, JC], F32, name="xic", tag="xic")
                nc.sync.dma_start(xic[:], xi[:, j0:j0 + JC])
                yic = wp.tile([128, JC], F32, name="yic", tag="yic")
                nc.sync.dma_start(yic[:], yi[:, j0:j0 + JC])
                zic = wp.tile([128, JC], F32, name="zic", tag="zic")
                nc.sync.dma_start(zic[:], zi[:, j0:j0 + JC])
                aicp = wp.tile([128, JC], F32, name="aicp", tag="aicp")
                nc.sync.dma_start(aicp[:], ai[:, j0:j0 + JC])
                gx = wp.tile([128, NI], F32, name="gx", tag="gx", bufs=1)
                nc.gpsimd.ap_gather(out_ap=gx[:], in_ap=tab[:], idxs_ap=idx[:],
                                    channels=128, num_elems=N, d=1, num_idxs=NI)

                XJ = wp.tile([128, JC], F32, tag="XJ")
                YJ = wp.tile([128, JC], F32, tag="YJ")
                ZJ = wp.tile([128, JC], F32, tag="ZJ")
                BJ = wp.tile([128, JC], F32, tag="BJ")
                for comp, dst in ((0, XJ), (1, YJ), (2, ZJ), (3, BJ)):
                    for k in range(NCORES):
                        src = gx[16 * k + comp:16 * k + comp + 1, :]
                        src3 = src.rearrange("p (s j) -> p s j", s=16)
                        nc.sync.dma_start(dst[16 * k:16 * k + 16, :], src3)

                def plane(tag):
                    return wp.tile([128, JC], F32, name=tag, tag=tag)

                ux, uy, uz = plane("ux"), plane("uy"), plane("uz")
                nc.vector.tensor_tensor(out=ux[:], in0=XJ[:], in1=xic[:], op=ALU.subtract)
                nc.vector.tensor_tensor(out=uy[:], in0=YJ[:], in1=yic[:], op=ALU.subtract)
                nc.vector.tensor_tensor(out=uz[:], in0=ZJ[:], in1=zic[:], op=ALU.subtract)
                g1 = plane("g1"); g2 = plane("g2"); km = plane("km")
                for u_ in (ux, uy, uz):
                    nc.vector.tensor_scalar(out=g1[:], in0=u_[:], scalar1=10.0,
                                            scalar2=None, op0=ALU.is_gt)
                    nc.vector.tensor_scalar(out=g2[:], in0=u_[:], scalar1=-10.0,
                                            scalar2=None, op0=ALU.is_lt)
                    nc.vector.tensor_tensor(out=km[:], in0=g1[:], in1=g2[:], op=ALU.subtract)
                    nc.vector.tensor_scalar(out=km[:], in0=km[:], scalar1=L,
                                            scalar2=None, op0=ALU.mult)
                    nc.vector.tensor_tensor(out=u_[:], in0=u_[:], in1=km[:], op=ALU.subtract)
                sqx, sqy, sqz = plane("sqx"), plane("sqy"), plane("sqz")
                nc.scalar.activation(sqx[:], ux[:], AF.Square)
                nc.scalar.activation(sqy[:], uy[:], AF.Square)
                nc.scalar.activation(sqz[:], uz[:], AF.Square)
                r2 = plane("r2")
                nc.vector.tensor_tensor(out=r2[:], in0=sqx[:], in1=sqy[:], op=ALU.add)
                nc.vector.tensor_tensor(out=r2[:], in0=r2[:], in1=sqz[:], op=ALU.add)
                r = plane("r")
                nc.scalar.activation(r[:], r2[:], AF.Sqrt, bias=1e-12)
                invr = plane("invr")
                nc.vector.reciprocal(invr[:], r[:])
                rc = plane("rc")
                nc.vector.tensor_scalar(out=rc[:], in0=r[:], scalar1=2.0,
                                        scalar2=None, op0=ALU.max)
                nc.vector.tensor_scalar(out=rc[:], in0=rc[:], scalar1=6.0,
                                        scalar2=None, op0=ALU.min)
                csw = plane("csw")
                nc.scalar.activation(csw[:], rc[:], AF.Sin,
                                     scale=float(-np.pi / 4), bias=float(np.pi))
                swp = plane("swp")
                nc.vector.tensor_scalar(out=swp[:], in0=csw[:], scalar1=0.5,
                                        scalar2=0.5, op0=ALU.mult, op1=ALU.add)
                v = plane("v")
                nc.vector.tensor_scalar(out=v[:], in0=nqc[:], scalar1=0,
                                        scalar2=None, op0=ALU.is_ge)
                vir = plane("vir")
                nc.vector.tensor_tensor(out=vir[:], in0=v[:], in1=invr[:], op=ALU.mult)
                s2 = plane("s2")
                nc.vector.tensor_tensor(out=s2[:], in0=swp[:], in1=vir[:], op=ALU.mult)
                w0 = plane("w0")
                nc.vector.tensor_tensor(out=w0[:], in0=s2[:], in1=invr[:], op=ALU.mult)

                lt = wp.tile([128, JC, 8], F32, tag="lt")
                nc.vector.memset(lt[:], 0.0)
                nc.vector.tensor_copy(out=lt[0:64, :, 0], in_=s2[0:64, :])
                nc.vector.tensor_copy(out=lt[64:128, :, 4], in_=s2[64:128, :])
                for di, u_ in enumerate((ux, uy, uz)):
                    rij = plane("rij")
                    nc.vector.tensor_tensor(out=rij[:], in0=u_[:], in1=w0[:], op=ALU.mult)
                    nc.vector.tensor_copy(out=lt[0:64, :, 1 + di], in_=rij[0:64, :])
                    nc.vector.tensor_copy(out=lt[64:128, :, 5 + di], in_=rij[64:128, :])

                # class-masked s and one planes (classes 0,1,2)
                aic = aicp[:]
                scls, ocls = {}, {}
                sa1, sB = plane("sa1"), plane("sB")
                nc.vector.tensor_tensor(out=sa1[:], in0=s2[:], in1=aic, op=ALU.mult)
                nc.vector.tensor_tensor(out=sB[:], in0=s2[:], in1=BJ[:], op=ALU.mult)
                scls[2], u1s, u2s = plane("sc2"), plane("u1s"), plane("u2s")
                nc.vector.tensor_tensor(out=scls[2][:], in0=sa1[:], in1=BJ[:], op=ALU.mult)
                nc.vector.tensor_tensor(out=u1s[:], in0=sa1[:], in1=scls[2][:], op=ALU.subtract)
                nc.vector.tensor_tensor(out=u2s[:], in0=sB[:], in1=scls[2][:], op=ALU.subtract)
                scls[1], t3s, scls[0] = plane("sc1"), plane("t3s"), plane("sc0")
                nc.vector.tensor_tensor(out=scls[1][:], in0=u1s[:], in1=u2s[:], op=ALU.add)
                nc.vector.tensor_tensor(out=t3s[:], in0=s2[:], in1=sa1[:], op=ALU.subtract)
                nc.vector.tensor_tensor(out=scls[0][:], in0=t3s[:], in1=u2s[:], op=ALU.subtract)
                oa1, oB = plane("oa1"), plane("oB")
                nc.vector.tensor_tensor(out=oa1[:], in0=v[:], in1=aic, op=ALU.mult)
                nc.vector.tensor_tensor(out=oB[:], in0=v[:], in1=BJ[:], op=ALU.mult)
                ocls[2], u1o, u2o = plane("oc2"), plane("u1o"), plane("u2o")
                nc.vector.tensor_tensor(out=ocls[2][:], in0=oa1[:], in1=BJ[:], op=ALU.mult)
                nc.vector.tensor_tensor(out=u1o[:], in0=oa1[:], in1=ocls[2][:], op=ALU.subtract)
                nc.vector.tensor_tensor(out=u2o[:], in0=oB[:], in1=ocls[2][:], op=ALU.subtract)
                ocls[1], t3o, ocls[0] = plane("oc1"), plane("t3o"), plane("oc0")
                nc.vector.tensor_tensor(out=ocls[1][:], in0=u1o[:], in1=u2o[:], op=ALU.add)
                nc.vector.tensor_tensor(out=t3o[:], in0=v[:], in1=oa1[:], op=ALU.subtract)
                nc.vector.tensor_tensor(out=ocls[0][:], in0=t3o[:], in1=u2o[:], op=ALU.subtract)

                for sub in range(JC // SUBJ):
                    jlo = sub * SUBJ
                    bas = bp.tile([128, SUBJ, Wb], F32, tag="bas")
                    for bi, (kind, ci, t) in enumerate(basis):
                        if kind == "one":
                            nc.scalar.copy(bas[:, :, bi], ocls[ci][:, jlo:jlo + SUBJ])
                        elif kind == "lin":
                            nc.vector.tensor_copy(out=bas[:, :, bi],
                                                  in_=scls[ci][:, jlo:jlo + SUBJ])
                        else:
                            nc.scalar.activation(bas[:, :, bi],
                                                 scls[ci][:, jlo:jlo + SUBJ],
                                                 AF.Relu, bias=float(-t))
                    for grp in range(SUBJ // 16):
                        phps = psp.tile([128, 128], F32, tag="phps")
                        for jj in range(16):
                            j = jlo + grp * 16 + jj
                            nc.tensor.matmul(out=phps[:Wb, jj * 8:(jj + 1) * 8],
                                             lhsT=bas[:, j - jlo, :],
                                             rhs=lt[:, j, :],
                                             start=True, stop=True)
                        phi = wp.tile([128, 128], F32, tag="phi")
                        if grp % 2 == 0:
                            nc.scalar.copy(phi[:Wb, :], phps[:Wb, :])
                        else:
                            nc.vector.tensor_copy(out=phi[:Wb, :], in_=phps[:Wb, :])
                        g_abs = (c * JC + jlo) // 16 + grp
                        qps = qsp.tile([128, 32], F32, tag="qps")
                        nc.tensor.matmul(out=qps[:], lhsT=phi[:Wb, :], rhs=w3s[:],
                                         start=True, stop=True)
                        if grp % 2 == 0:
                            nc.vector.tensor_copy(
                                out=qt[:, g_abs * 32:(g_abs + 1) * 32], in_=qps[:])
                        else:
                            nc.scalar.copy(qt[:, g_abs * 32:(g_abs + 1) * 32], qps[:])

            # Q relayout: qt[8*j16+4*q+d, 32*gp+g] -> q2[32*(gp%4)+2*j16+q, 128*(gp//4)+32*d+g]
            qtv = qt[:].rearrange("p (gp g) -> p gp g", g=32)
            q2v = q2[:].rearrange("p (t d g) -> p t d g", d=4, g=32)
            for qq in range(2):
                for d in range(4):
                    for k4 in range(4):
                        src = qtv[4 * qq + d::8, k4::4, :]                 # [16, 32, 32]
                        dst = q2v[32 * k4 + qq:32 * k4 + qq + 31:2, :, d, :]
                        nc.sync.dma_start(dst, src)

            # D stage
            for t in range(32):
                acc = wp.tile([128, 512], F32, tag="dacc")
                tmp = wp.tile([128, 512], F32, tag="dtmp")
                for d in range(4):
                    off = 128 * t + 32 * d
                    qg = q2[:, off:off + 32]
                    in0 = qg.to_broadcast([128, 32, 16])
                    qk = q2[:, off:off + 16]
                    in1 = bass.AP(qk.tensor, qk.offset, [[4096, 128], [0, 32], [1, 16]])
                    dstv = (acc if d == 0 else tmp)[:].rearrange("p (g k) -> p g k", k=16)
                    nc.vector.tensor_tensor(out=dstv, in0=in0, in1=in1, op=ALU.mult)
                    if d > 0:
                        nc.vector.tensor_tensor(out=acc[:], in0=acc[:], in1=tmp[:], op=ALU.add)
                nc.sync.dma_start(dout[128 * t:128 * (t + 1), :], acc[:])

    nc.compile()
    return nc


def _prep_core(pos, types, neigh):
    comp = np.empty((4, N), np.float32)
    comp[0], comp[1], comp[2] = pos[:, 0], pos[:, 1], pos[:, 2]
    comp[3] = types.astype(np.float32)
    table = np.empty((128, N), np.float32)
    for p in range(128):
        table[p] = comp[p % 4]

    nv = neigh.reshape(JTOT, 2, M)
    nq = np.ascontiguousarray(nv.transpose(1, 2, 0).reshape(128, JTOT)).astype(np.int32)

    idxw = np.empty((128, JTOT), np.int16)
    nq_cl = np.maximum(nq, 0).astype(np.int16)
    for c in range(NCHUNK):
        blk = nq_cl[:, c * JC:(c + 1) * JC]
        for k in range(NCORES):
            stream = blk[16 * k:16 * k + 16, :].reshape(16 * JC)    # i = s*JC + j
            wrapped = stream.reshape(JC, 16).T                       # [p, cc]
            idxw[16 * k:16 * k + 16, c * JC:(c + 1) * JC] = wrapped

    par = pos.reshape(JTOT, 2, 3)
    def repl(x):  # [2, JTOT] -> [128, JTOT]
        return np.ascontiguousarray(
            np.broadcast_to(x[:, None, :], (2, M, JTOT)).reshape(128, JTOT)
        ).astype(np.float32)
    xi = repl(par[:, :, 0].T)
    yi = repl(par[:, :, 1].T)
    zi = repl(par[:, :, 2].T)
    ai = repl(types.reshape(JTOT, 2).T.astype(np.float32))
    return dict(table=table, idxw=idxw, nq=nq, xi=xi, yi=yi, zi=zi, ai=ai)


_CACHE = {}


def kernel(**inputs):
    inputs = {k: np.asarray(v) for k, v in inputs.items()}
    ws = {k: inputs[k].astype(np.float32) for k in
          ("es1_w", "es1_b", "es2_w", "es2_b", "fs1_w", "fs1_b", "fs2_w", "fs2_b",
           "en1_w", "en1_b", "en2_w", "en2_b", "en3_w", "en3_b")}
    key = hash(tuple(ws[k].tobytes() for k in sorted(ws)))
    if key not in _CACHE:
        basis, w3pp = _fold_weights(ws)
        _verify_fold(ws, basis, w3pp)
        nc = _build_program(basis)
        _CACHE[key] = (w3pp, nc)
    w3pp, nc = _CACHE[key]

    pos = inputs["inputs"].astype(np.float32)
    types = inputs["input_types"].astype(np.int64)
    neigh = inputs["neigh_list"].astype(np.int64)

    in_maps = []
    for s in range(S):
        m = _prep_core(pos[s], types[s], neigh[s])
        m["w3pp"] = w3pp
        in_maps.append(m)

    res = run_bass_kernel_spmd(nc, in_maps, core_ids=list(range(NCORES)))
    out = np.stack([r["dout"].reshape(N, 32, 16) for r in res.results], 0)
    return out.astype(np.float32)



# revision 4
# speedup vs baseline: 11.8254x; 11.8254x over previous
"""Trainium2 Bass kernel for nn_DescriptorModuleSpecies (gnn_message_passing).

Sharding: data-parallel, one snapshot per NeuronCore (8 cores).

Algorithmic core (exact algebra of the reference):
    D[n] = Q[n]^T @ Q[n][:, :16],   Q[n] = sum_m r_tilde(n,m) (x) G(s(n,m), pair)
The species-pair MLPs (es/fs) and en1/en2 are folded on the host into an
exact piecewise-linear basis in s per species-pair class:
    G(s; class) = sum_beta phi_beta(s) * W3''[beta, :]      (W ~= 54 basis fns)
Per-edge basis planes cost one fused DVE/ACT op each; per-atom moments
Phi[d, beta] = sum_m r_tilde_d * phi_beta are computed with one small PE
matmul per atom-pair column, then Q = Phi @ W3''.

I/O is minimized for the axon tunnel: the device receives only a compact
component table comp[4, N+1] (x, y, z, type+1 with a zero sentinel column
for padded neighbor slots) plus int16 per-Q7-core gather index streams, and
returns Q in fp16 ([128, 4096] per core). The table is replicated to 128
partitions on-device with doubling DMAs; self-atom planes are produced by
appending self indices to the gather streams and broadcasting the gathered
row to 16 partitions with a stride-0-read DMA. D = Q^T Q is reconstructed
on the host (|Q| <= sqrt(max |D|) ~ 17, so fp16 is far inside the 2e-2
tolerance).
"""

import sys

import numpy as np

try:
    import concourse.bass as bass  # noqa: F401
except Exception:  # pragma: no cover
    sys.path.insert(0, "/opt/trn_rl_repo")

import concourse.bass as bass
import concourse.bacc as bacc
import concourse.mybir as mybir
from concourse._compat import axon_active
from concourse.bass_utils import run_bass_kernel_spmd
from concourse.tile import TileContext

F32 = mybir.dt.float32
F16 = mybir.dt.float16
I32 = mybir.dt.int32
I16 = mybir.dt.int16
AF = mybir.ActivationFunctionType
ALU = mybir.AluOpType

S, N, M = 8, 4096, 64
L = 20.0
NT = N + 1                 # table columns: col 0 = sentinel, col 1+a = atom a
JTOT = N // 2              # 2048 atom-pair columns
NCHUNK = 8
JC = JTOT // NCHUNK        # 256 cols per chunk
NI = 16 * JC               # neighbor gather idxs per Q7 core per chunk
NI2 = NI + JC              # + self-atom idxs
IW = NI2 // 16             # wrapped idx columns per chunk (272)
NCORES = 8
SUBJ = 128                 # moment sub-chunk (j columns per bas tile)

CLASSES = [(0, 0), (0, 1), (1, 1)]   # pair (0,1) == (1,0) exactly (symmetrized)


def _mlp_np(x, params):
    n = len(params)
    for i, (w, b) in enumerate(params):
        x = x @ w + b
        if i < n - 1:
            x = np.maximum(x, 0.0)
    return x


def _fold_weights(ws):
    """Exact PL basis for h2(s; class) folded with en3 into W3''.

    Returns (basis, w3pp): basis is a list of ("one"|"lin"|"relu", cls, knot);
    w3pp [W, 32] f32 with G_edge = sum_beta basis_beta * w3pp[beta]."""
    es = [(ws["es1_w"], ws["es1_b"]), (ws["es2_w"], ws["es2_b"])]
    fs = [(ws["fs1_w"], ws["fs1_b"]), (ws["fs2_w"], ws["fs2_b"])]
    W1, b1 = ws["en1_w"].astype(np.float64), ws["en1_b"].astype(np.float64)
    W2, b2 = ws["en2_w"].astype(np.float64), ws["en2_b"].astype(np.float64)
    W3, b3 = ws["en3_w"].astype(np.float64), ws["en3_b"].astype(np.float64)

    basis, psis = [], []
    for ci, (a, b) in enumerate(CLASSES):
        pair = np.array([[a, b]], dtype=np.float32)
        td = _mlp_np(_mlp_np(pair, es) + _mlp_np(pair[:, ::-1], es), fs)[0]
        td = td.astype(np.float64)
        U = td @ W1                                   # [8]

        def h2_of(s):
            h1 = np.maximum(np.outer(s, U) + b1[None, :], 0.0)
            return np.maximum(h1 @ W2 + b2[None, :], 0.0)

        kn1 = sorted(float(-b1[c] / U[c]) for c in range(8)
                     if U[c] != 0.0 and -b1[c] / U[c] > 0.0)
        segpts = [0.0] + kn1
        cross = set()
        for i in range(len(segpts)):
            lo = segpts[i]
            hi = segpts[i + 1] if i + 1 < len(segpts) else None
            mid = (lo + hi) / 2 if hi is not None else lo + 1.0
            act = (mid * U + b1) > 0
            z_lo = np.maximum(lo * U + b1, 0.0) @ W2 + b2
            slope = (U * act) @ W2
            for f in range(16):
                if slope[f] == 0.0:
                    continue
                t = lo - z_lo[f] / slope[f]
                if t > lo and (hi is None or t < hi) and t > 0.0:
                    cross.add(float(t))
        knots = sorted(set(kn1) | cross)

        def seg_slope(lo, hi):
            mid = (lo + hi) / 2 if hi is not None else lo + 1.0
            act1 = (mid * U + b1) > 0
            z_mid = np.maximum(mid * U + b1, 0.0) @ W2 + b2
            return ((U * act1) @ W2) * (z_mid > 0)

        alpha = h2_of(np.array([0.0]))[0]
        bounds = knots + [None]
        slopes = [seg_slope(0.0 if i == 0 else knots[i - 1], bounds[i])
                  for i in range(len(knots) + 1)]
        basis.append(("one", ci, 0.0)); psis.append(alpha)
        basis.append(("lin", ci, 0.0)); psis.append(slopes[0])
        for i, t in enumerate(knots):
            basis.append(("relu", ci, float(t)))
            psis.append(slopes[i + 1] - slopes[i])

    Psi = np.stack(psis, 0)
    w3pp = Psi @ W3
    for i, (kind, ci, t) in enumerate(basis):
        if kind == "one":
            w3pp[i] += b3
    return basis, w3pp.astype(np.float32)


def _verify_fold(ws, basis, w3pp):
    es = [(ws["es1_w"], ws["es1_b"]), (ws["es2_w"], ws["es2_b"])]
    fs = [(ws["fs1_w"], ws["fs1_b"]), (ws["fs2_w"], ws["fs2_b"])]
    rng = np.random.default_rng(0)
    sv = np.concatenate([rng.uniform(0, 5, 64), rng.uniform(0, 1000, 32), [0.0]])
    for ci, (a, b) in enumerate(CLASSES):
        pair = np.array([[a, b]], dtype=np.float32)
        td = _mlp_np(_mlp_np(pair, es) + _mlp_np(pair[:, ::-1], es), fs)[0]
        st = sv[:, None] * td[None, :].astype(np.float64)
        G = _mlp_np(st, [(ws["en1_w"], ws["en1_b"]), (ws["en2_w"], ws["en2_b"]),
                         (ws["en3_w"], ws["en3_b"])])
        vals = np.zeros((len(sv), len(basis)))
        for i, (kind, cc, t) in enumerate(basis):
            if cc != ci:
                continue
            vals[:, i] = 1.0 if kind == "one" else (sv if kind == "lin"
                                                    else np.maximum(sv - t, 0.0))
        Gb = vals @ w3pp.astype(np.float64)
        err = np.abs(Gb - G).max() / (np.abs(G).max() + 1e-9)
        assert err < 1e-4, f"basis fold mismatch class {ci}: rel {err}"


def _reg_consts(nc, vals):
    for v in vals:
        key = (F32, float(v))
        if key in nc.const_aps.aps:
            continue
        t = nc.alloc_sbuf_tensor(f"constf32_{len(nc.const_aps.aps)}", [128, 1], F32)
        nc.gpsimd.memset(t.ap(), float(v))
        nc.const_aps.aps[key] = t.ap()
    nc.all_engine_barrier()


def _build_program(basis):
    Wb = len(basis)
    assert Wb <= 128
    act_knots = sorted({t for k, c, t in basis if k == "relu"})

    nc = bacc.Bacc("TRN2", target_bir_lowering=False, debug=False,
                   num_devices=NCORES)
    _reg_consts(nc, [0.0, 1e-12, float(np.pi)] + [-t for t in act_knots])

    comp_d = nc.dram_tensor("comp", [4, NT], F32, kind="ExternalInput")
    idxw = nc.dram_tensor("idxw", [128, NCHUNK * IW], I16, kind="ExternalInput")
    w3t = nc.dram_tensor("w3pp", [Wb, 32], F32, kind="ExternalInput")
    qout = nc.dram_tensor("qout", [128, 4096], F16, kind="ExternalOutput")

    with TileContext(nc) as tc:
        with (
            tc.tile_pool(name="persist", bufs=1) as pp,
            tc.tile_pool(name="work", bufs=2) as wp,
            tc.tile_pool(name="bas", bufs=1) as bp,
            tc.tile_pool(name="psum", bufs=4, space="PSUM") as psp,
            tc.tile_pool(name="qpsum", bufs=4, space="PSUM") as qsp,
        ):
            # replicated component table: row p holds comp[p % 4]
            tab = pp.tile([128, NT], F32)
            nc.sync.dma_start(tab[0:4, :], comp_d[:])
            for r in (4, 8, 16, 32, 64):
                nc.sync.dma_start(tab[r:2 * r, :], tab[0:r, :])
            w3s = pp.tile([Wb, 32], F32)
            nc.sync.dma_start(w3s[:], w3t[:])
            qt = pp.tile([128, 4096], F16)     # [(8j16+4q+d), 32*grp + g]

            for c in range(NCHUNK):
                idx = wp.tile([128, IW], I16, tag="idx")
                nc.sync.dma_start(idx[:], idxw[:, c * IW:(c + 1) * IW])
                gx = wp.tile([128, NI2], F32, name="gx", tag="gx", bufs=1)
                nc.gpsimd.ap_gather(out_ap=gx[:], in_ap=tab[:], idxs_ap=idx[:],
                                    channels=128, num_elems=NT, d=1, num_idxs=NI2)

                XJ = wp.tile([128, JC], F32, tag="XJ")
                YJ = wp.tile([128, JC], F32, tag="YJ")
                ZJ = wp.tile([128, JC], F32, tag="ZJ")
                BJ = wp.tile([128, JC], F32, tag="BJ")
                xic = wp.tile([128, JC], F32, tag="xic")
                yic = wp.tile([128, JC], F32, tag="yic")
                zic = wp.tile([128, JC], F32, tag="zic")
                aicp = wp.tile([128, JC], F32, tag="aicp")
                for comp, dst, dsts in ((0, XJ, xic), (1, YJ, yic),
                                        (2, ZJ, zic), (3, BJ, aicp)):
                    for k in range(NCORES):
                        src = gx[16 * k + comp:16 * k + comp + 1, 0:NI]
                        src3 = src.rearrange("p (s j) -> p s j", s=16)
                        nc.sync.dma_start(dst[16 * k:16 * k + 16, :], src3)
                        srcs = gx[16 * k + comp:16 * k + comp + 1, NI:NI2]
                        srcb = bass.AP(srcs.tensor, srcs.offset,
                                       [[NI2, 1], [0, 16], [1, JC]])
                        nc.sync.dma_start(dsts[16 * k:16 * k + 16, :], srcb)

                def plane(tag):
                    return wp.tile([128, JC], F32, name=tag, tag=tag)

                ux, uy, uz = plane("ux"), plane("uy"), plane("uz")
                nc.vector.tensor_tensor(out=ux[:], in0=XJ[:], in1=xic[:], op=ALU.subtract)
                nc.vector.tensor_tensor(out=uy[:], in0=YJ[:], in1=yic[:], op=ALU.subtract)
                nc.vector.tensor_tensor(out=uz[:], in0=ZJ[:], in1=zic[:], op=ALU.subtract)
                g1 = plane("g1"); g2 = plane("g2"); km = plane("km")
                for u_ in (ux, uy, uz):
                    nc.vector.tensor_scalar(out=g1[:], in0=u_[:], scalar1=10.0,
                                            scalar2=None, op0=ALU.is_gt)
                    nc.vector.tensor_scalar(out=g2[:], in0=u_[:], scalar1=-10.0,
                                            scalar2=None, op0=ALU.is_lt)
                    nc.vector.tensor_tensor(out=km[:], in0=g1[:], in1=g2[:], op=ALU.subtract)
                    nc.vector.tensor_scalar(out=km[:], in0=km[:], scalar1=L,
                                            scalar2=None, op0=ALU.mult)
                    nc.vector.tensor_tensor(out=u_[:], in0=u_[:], in1=km[:], op=ALU.subtract)
                sqx, sqy, sqz = plane("sqx"), plane("sqy"), plane("sqz")
                nc.scalar.activation(sqx[:], ux[:], AF.Square)
                nc.scalar.activation(sqy[:], uy[:], AF.Square)
                nc.scalar.activation(sqz[:], uz[:], AF.Square)
                r2 = plane("r2")
                nc.vector.tensor_tensor(out=r2[:], in0=sqx[:], in1=sqy[:], op=ALU.add)
                nc.vector.tensor_tensor(out=r2[:], in0=r2[:], in1=sqz[:], op=ALU.add)
                r = plane("r")
                nc.scalar.activation(r[:], r2[:], AF.Sqrt, bias=1e-12)
                invr = plane("invr")
                nc.vector.reciprocal(invr[:], r[:])
                rc = plane("rc")
                nc.vector.tensor_scalar(out=rc[:], in0=r[:], scalar1=2.0,
                                        scalar2=None, op0=ALU.max)
                nc.vector.tensor_scalar(out=rc[:], in0=rc[:], scalar1=6.0,
                                        scalar2=None, op0=ALU.min)
                csw = plane("csw")
                nc.scalar.activation(csw[:], rc[:], AF.Sin,
                                     scale=float(-np.pi / 4), bias=float(np.pi))
                swp = plane("swp")
                nc.vector.tensor_scalar(out=swp[:], in0=csw[:], scalar1=0.5,
                                        scalar2=0.5, op0=ALU.mult, op1=ALU.add)
                v = plane("v")
                nc.vector.tensor_scalar(out=v[:], in0=BJ[:], scalar1=0.5,
                                        scalar2=None, op0=ALU.is_ge)
                aic = plane("aic")
                nc.vector.tensor_scalar(out=aic[:], in0=aicp[:], scalar1=1.0,
                                        scalar2=None, op0=ALU.subtract)
                bjt = plane("bjt")
                nc.vector.tensor_scalar(out=bjt[:], in0=BJ[:], scalar1=1.0,
                                        scalar2=None, op0=ALU.subtract)
                vir = plane("vir")
                nc.vector.tensor_tensor(out=vir[:], in0=v[:], in1=invr[:], op=ALU.mult)
                s2 = plane("s2")
                nc.vector.tensor_tensor(out=s2[:], in0=swp[:], in1=vir[:], op=ALU.mult)
                w0 = plane("w0")
                nc.vector.tensor_tensor(out=w0[:], in0=s2[:], in1=invr[:], op=ALU.mult)

                lt = wp.tile([128, JC, 8], F32, tag="lt")
                nc.vector.memset(lt[:], 0.0)
                nc.vector.tensor_copy(out=lt[0:64, :, 0], in_=s2[0:64, :])
                nc.vector.tensor_copy(out=lt[64:128, :, 4], in_=s2[64:128, :])
                for di, u_ in enumerate((ux, uy, uz)):
                    rij = plane("rij")
                    nc.vector.tensor_tensor(out=rij[:], in0=u_[:], in1=w0[:], op=ALU.mult)
                    nc.vector.tensor_copy(out=lt[0:64, :, 1 + di], in_=rij[0:64, :])
                    nc.vector.tensor_copy(out=lt[64:128, :, 5 + di], in_=rij[64:128, :])

                # class-masked s and one planes (classes 0,1,2)
                scls, ocls = {}, {}
                sa1, sB = plane("sa1"), plane("sB")
                nc.vector.tensor_tensor(out=sa1[:], in0=s2[:], in1=aic[:], op=ALU.mult)
                nc.vector.tensor_tensor(out=sB[:], in0=s2[:], in1=bjt[:], op=ALU.mult)
                scls[2], u1s, u2s = plane("sc2"), plane("u1s"), plane("u2s")
                nc.vector.tensor_tensor(out=scls[2][:], in0=sa1[:], in1=bjt[:], op=ALU.mult)
                nc.vector.tensor_tensor(out=u1s[:], in0=sa1[:], in1=scls[2][:], op=ALU.subtract)
                nc.vector.tensor_tensor(out=u2s[:], in0=sB[:], in1=scls[2][:], op=ALU.subtract)
                scls[1], t3s, scls[0] = plane("sc1"), plane("t3s"), plane("sc0")
                nc.vector.tensor_tensor(out=scls[1][:], in0=u1s[:], in1=u2s[:], op=ALU.add)
                nc.vector.tensor_tensor(out=t3s[:], in0=s2[:], in1=sa1[:], op=ALU.subtract)
                nc.vector.tensor_tensor(out=scls[0][:], in0=t3s[:], in1=u2s[:], op=ALU.subtract)
                oa1, oB = plane("oa1"), plane("oB")
                nc.vector.tensor_tensor(out=oa1[:], in0=v[:], in1=aic[:], op=ALU.mult)
                nc.vector.tensor_tensor(out=oB[:], in0=v[:], in1=bjt[:], op=ALU.mult)
                ocls[2], u1o, u2o = plane("oc2"), plane("u1o"), plane("u2o")
                nc.vector.tensor_tensor(out=ocls[2][:], in0=oa1[:], in1=bjt[:], op=ALU.mult)
                nc.vector.tensor_tensor(out=u1o[:], in0=oa1[:], in1=ocls[2][:], op=ALU.subtract)
                nc.vector.tensor_tensor(out=u2o[:], in0=oB[:], in1=ocls[2][:], op=ALU.subtract)
                ocls[1], t3o, ocls[0] = plane("oc1"), plane("t3o"), plane("oc0")
                nc.vector.tensor_tensor(out=ocls[1][:], in0=u1o[:], in1=u2o[:], op=ALU.add)
                nc.vector.tensor_tensor(out=t3o[:], in0=v[:], in1=oa1[:], op=ALU.subtract)
                nc.vector.tensor_tensor(out=ocls[0][:], in0=t3o[:], in1=u2o[:], op=ALU.subtract)

                for sub in range(JC // SUBJ):
                    jlo = sub * SUBJ
                    bas = bp.tile([128, SUBJ, Wb], F32, tag="bas")
                    for bi, (kind, ci, t) in enumerate(basis):
                        if kind == "one":
                            nc.scalar.copy(bas[:, :, bi], ocls[ci][:, jlo:jlo + SUBJ])
                        elif kind == "lin":
                            nc.vector.tensor_copy(out=bas[:, :, bi],
                                                  in_=scls[ci][:, jlo:jlo + SUBJ])
                        else:
                            nc.scalar.activation(bas[:, :, bi],
                                                 scls[ci][:, jlo:jlo + SUBJ],
                                                 AF.Relu, bias=float(-t))
                    for grp in range(SUBJ // 16):
                        phps = psp.tile([128, 128], F32, tag="phps")
                        for jj in range(16):
                            j = jlo + grp * 16 + jj
                            nc.tensor.matmul(out=phps[:Wb, jj * 8:(jj + 1) * 8],
                                             lhsT=bas[:, j - jlo, :],
                                             rhs=lt[:, j, :],
                                             start=True, stop=True)
                        phi = wp.tile([128, 128], F32, tag="phi")
                        if grp % 2 == 0:
                            nc.scalar.copy(phi[:Wb, :], phps[:Wb, :])
                        else:
                            nc.vector.tensor_copy(out=phi[:Wb, :], in_=phps[:Wb, :])
                        g_abs = (c * JC + jlo) // 16 + grp
                        qps = qsp.tile([128, 32], F32, tag="qps")
                        nc.tensor.matmul(out=qps[:], lhsT=phi[:Wb, :], rhs=w3s[:],
                                         start=True, stop=True)
                        if grp % 2 == 0:
                            nc.vector.tensor_copy(
                                out=qt[:, g_abs * 32:(g_abs + 1) * 32], in_=qps[:])
                        else:
                            nc.scalar.copy(qt[:, g_abs * 32:(g_abs + 1) * 32], qps[:])

                nc.sync.dma_start(qout[:, c * 512:(c + 1) * 512],
                                  qt[:, c * 512:(c + 1) * 512])

    nc.compile()
    return nc


def _static_self_part():
    # selfpart[k, p, c, w] = 2*(JC*c + w*16 + p) + (k//4) + 1
    k = np.arange(8)[:, None, None, None]
    p = np.arange(16)[None, :, None, None]
    c = np.arange(NCHUNK)[None, None, :, None]
    w = np.arange(16)[None, None, None, :]
    return (2 * (JC * c + w * 16 + p) + (k // 4) + 1).astype(np.int16)


_SELF_PART = _static_self_part()


def _prep_core(pos, types, neigh):
    comp = np.zeros((4, NT), np.float32)
    comp[0:3, 1:] = pos.T
    comp[3, 1:] = types + 1.0

    # neighbor indices shifted +1 so pads (-1) hit the zero sentinel column
    nq16 = (neigh + 1).astype(np.int16)                       # [N, M]
    nq = np.ascontiguousarray(
        nq16.reshape(JTOT, 2, M).transpose(1, 2, 0)).reshape(128, JTOT)
    X2 = nq.reshape(8, 16, NCHUNK, 16, 16)                    # [k, s, c, t, p]
    W = np.empty((8, 16, NCHUNK, IW), np.int16)
    W[:, :, :, :256] = X2.transpose(0, 4, 2, 1, 3).reshape(8, 16, NCHUNK, 256)
    W[:, :, :, 256:] = _SELF_PART
    return dict(comp=comp, idxw=W.reshape(128, NCHUNK * IW))


_CACHE = {}
_RUNNER = {}


def _make_runner(nc):
    """Cached shard_map-jitted executor for the axon/PJRT path (avoids the
    per-call retrace+recompile of run_bass_kernel_spmd)."""
    import jax
    from jax.sharding import Mesh, PartitionSpec
    try:
        from jax import shard_map as _shard_map
    except ImportError:
        from jax.experimental.shard_map import shard_map as _shard_map

    def shard_map(f, **kw):
        try:
            return _shard_map(f, **kw, check_vma=False)
        except TypeError:
            return _shard_map(f, **kw, check_rep=False)

    from concourse import bass2jax

    bass2jax.install_neuronx_cc_hook()
    partition_name = nc.partition_id_tensor.name if nc.partition_id_tensor else None
    in_names, out_names, out_avals = [], [], []
    for alloc in nc.m.functions[0].allocations:
        if not isinstance(alloc, mybir.MemoryLocationSet):
            continue
        name = alloc.memorylocations[0].name
        if alloc.kind == "ExternalInput":
            if name != partition_name:
                in_names.append(name)
        elif alloc.kind == "ExternalOutput":
            out_names.append(name)
            out_avals.append(jax.core.ShapedArray(
                tuple(alloc.tensor_shape), mybir.dt.np(alloc.dtype)))
    n_params = len(in_names)
    n_outs = len(out_avals)
    bind_names = in_names + out_names + ([partition_name] if partition_name else [])
    donate = tuple(range(n_params, n_params + n_outs))

    def _body(*args):
        operands = list(args)
        if partition_name is not None:
            operands.append(bass2jax.partition_id_tensor())
        outs = bass2jax._bass_exec_p.bind(
            *operands, out_avals=tuple(out_avals), in_names=tuple(bind_names),
            out_names=tuple(out_names), lowering_input_output_aliases=(),
            sim_require_finite=True, sim_require_nnan=True, nc=nc)
        return tuple(outs)

    devices = jax.devices()[:NCORES]
    mesh = Mesh(np.asarray(devices), ("core",))
    sharded = jax.jit(
        shard_map(_body, mesh=mesh,
                  in_specs=(PartitionSpec("core"),) * (n_params + n_outs),
                  out_specs=(PartitionSpec("core"),) * n_outs),
        donate_argnums=donate, keep_unused=True)

    def run(in_maps):
        concat_in = [
            np.concatenate([np.asarray(m[name]) for m in in_maps], axis=0)
            for name in in_names
        ]
        concat_zeros = [
            np.zeros((NCORES * a.shape[0], *a.shape[1:]), a.dtype)
            for a in out_avals
        ]
        out_arrs = sharded(*concat_in, *concat_zeros)
        return [
            {name: np.asarray(out_arrs[i]).reshape(NCORES, *out_avals[i].shape)[c]
             for i, name in enumerate(out_names)}
            for c in range(NCORES)
        ]

    return run


def _run(nc, in_maps):
    if axon_active():
        key = id(nc)
        if key not in _RUNNER:
            _RUNNER[key] = _make_runner(nc)
        return _RUNNER[key](in_maps)
    res = run_bass_kernel_spmd(nc, in_maps, core_ids=list(range(NCORES)))
    return res.results


def _q_from_qt(q16):
    """[128, 4096] f16 -> Q [N, 4, 32] f32 (n = 2*j + q)."""
    A = q16.astype(np.float32).reshape(16, 8, 128, 32)        # [jj, e, gabs, g]
    return np.ascontiguousarray(
        A.transpose(2, 0, 1, 3)).reshape(N, 4, 32)


def _host_d(qts):
    """qts: [S, 128, 4096] f16 -> D [S, N, 32, 16] f32 on host."""
    Q = np.stack([_q_from_qt(q) for q in qts], 0).reshape(S * N, 4, 32)
    try:
        import jax
        import jax.numpy as jnp

        if "d_fn" not in _RUNNER:
            def _d(q):
                return jnp.einsum("ndg,ndk->ngk", q, q[:, :, :16])
            cpu = jax.devices("cpu")[0]
            _RUNNER["d_fn"] = jax.jit(_d, device=cpu)
        D = np.asarray(_RUNNER["d_fn"](Q))
    except Exception:
        D = np.zeros((S * N, 32, 16), np.float32)
        for d in range(4):
            D += Q[:, d, :, None] * Q[:, d, None, :16]
    return D.reshape(S, N, 32, 16).astype(np.float32)


def kernel(**inputs):
    inputs = {k: np.asarray(v) for k, v in inputs.items()}
    ws = {k: inputs[k].astype(np.float32) for k in
          ("es1_w", "es1_b", "es2_w", "es2_b", "fs1_w", "fs1_b", "fs2_w", "fs2_b",
           "en1_w", "en1_b", "en2_w", "en2_b", "en3_w", "en3_b")}
    key = hash(tuple(ws[k].tobytes() for k in sorted(ws)))
    if key not in _CACHE:
        basis, w3pp = _fold_weights(ws)
        _verify_fold(ws, basis, w3pp)
        nc = _build_program(basis)
        _CACHE[key] = (w3pp, nc)
    w3pp, nc = _CACHE[key]

    pos = inputs["inputs"].astype(np.float32)
    types = inputs["input_types"].astype(np.int64)
    neigh = inputs["neigh_list"].astype(np.int64)

    in_maps = []
    for s in range(S):
        m = _prep_core(pos[s], types[s], neigh[s])
        m["w3pp"] = w3pp
        in_maps.append(m)

    results = _run(nc, in_maps)
    qts = [r["qout"] for r in results]
    return _host_d(qts)
